# revision 3
# baseline (speedup 1.0000x reference)
"""Causal self-attention (GQA + RMSNorm + RoPE) Trainium2 Bass kernel.

Sharding: data-parallel over (batch, seq-half). 8 cores = 4 batches x 2
sequence halves. Each core computes full K/V for its batch (cheap) and
queries for 1024 rows chosen as a balanced pair of 512-row blocks
(half 0 -> abs blocks {0, 3}, half 1 -> {1, 2}) so causal work is even.
No collectives: output row-slices are disjoint and gathered on host.

On-chip layout is channel-major (transposed): all host-side transposes
(x.T, W.T) are free; attention runs as S^T = K^T.T @ Q^T with softmax
sums via a ones-vector matmul on the PE, and normalization deferred to
the attention output (y^T tiles scaled by 1/sum broadcast). Causal
masking is a bf16 {0,1} multiply after exp (scores are rms-bounded, so
unmasked exp never overflows).

All matmul operands are bf16 (full PE rate, fp32 PSUM accumulation);
stats/softmax math stays fp32. K/V/Y stay SBUF-resident between phases;
only Q round-trips through DRAM (its latency hides behind the K/V
phase). Wk/Wv/cos/sin prefetch during phase Q.
"""

import os
import sys

sys.path.insert(0, "/opt/trn_rl_repo")

import ml_dtypes
import numpy as np

DIM = 2048
H = 16
HKV = 4
HD = 128
REP = H // HKV
B = 4
T = 2048
RB = 512          # rows per q slot
NSLOT = 2
R = RB * NSLOT    # 1024 q rows per core
EXT = (1024, 2048)  # key extent per slot (compile-time, covers both core halves)
DT = DIM // 128   # 16 contraction tiles
NJT = T // 128    # 16 key tiles
ROPE_BASE = 10000.0
EPS = float(np.finfo(np.float32).eps)
BF16 = ml_dtypes.bfloat16

_CACHE = {}


def _build():
    """Build + compile the SPMD Bass program (once per process)."""
    from concourse import bacc
    import concourse.mybir as mybir
    import concourse.tile as tile

    F32 = mybir.dt.float32
    BF = mybir.dt.bfloat16
    AF = mybir.ActivationFunctionType

    nc = bacc.Bacc("TRN2", target_bir_lowering=False, debug=False)

    xT = nc.dram_tensor("xT", [DIM, T], BF, kind="ExternalInput")
    xqT = nc.dram_tensor("xqT", [DIM, R], BF, kind="ExternalInput")
    WqT = nc.dram_tensor("WqT", [DIM, DIM], BF, kind="ExternalInput")
    WkT = nc.dram_tensor("WkT", [DIM, HKV * HD], BF, kind="ExternalInput")
    WvT = nc.dram_tensor("WvT", [DIM, HKV * HD], BF, kind="ExternalInput")
    WpT = nc.dram_tensor("WpT", [DIM, DIM], BF, kind="ExternalInput")
    qgain = nc.dram_tensor("qgain", [H], F32, kind="ExternalInput")
    cosq = nc.dram_tensor("cosq", [HD, R], F32, kind="ExternalInput")
    sinq = nc.dram_tensor("sinq", [HD, R], F32, kind="ExternalInput")
    cosk = nc.dram_tensor("cosk", [HD, T], F32, kind="ExternalInput")
    sink = nc.dram_tensor("sink", [HD, T], F32, kind="ExternalInput")
    # multiplicative {0,1} masks, key-major: m0 = slot0 vs keys [0,1024),
    # m1 = slot1 vs keys [1024,2048). slot1 keys [0,1024) are always visible.
    m0 = nc.dram_tensor("m0", [EXT[0], RB], BF, kind="ExternalInput")
    m1 = nc.dram_tensor("m1", [EXT[0], RB], BF, kind="ExternalInput")
    outT = nc.dram_tensor("outT", [DIM, R], F32, kind="ExternalOutput")

    qTd = nc.dram_tensor("qTd", [H, HD, R], BF, kind="Internal")

    with tile.TileContext(nc) as tc:
        with tc.tile_pool(name="const", bufs=1) as constp, \
             tc.tile_pool(name="tmp", bufs=3) as tmpp, \
             tc.tile_pool(name="kgv", bufs=1) as kgvp:
            ones = constp.tile([128, 1], BF)
            nc.vector.memset(ones, 1.0)
            qg = constp.tile([1, H], F32)
            nc.sync.dma_start(out=qg, in_=qgain[None, :])
            epsq = constp.tile([1, 1], F32)
            nc.vector.memset(epsq, EPS * HD)   # q scale: 1/sqrt(ssum + HD*eps)
            epsk = constp.tile([1, 1], F32)
            nc.vector.memset(epsk, EPS)        # k scale: rsqrt(ssum/HD + eps)

            # K^T, V, Y stay resident in SBUF across phases (no DRAM trips)
            kg_all = kgvp.tile([128, HKV, T], BF)     # [hd, g, j]
            v_all = kgvp.tile([128, NJT, HKV * HD], BF)  # [j%128, jtile, c]

            # rms-normalize the PSUM tile qkT [128, 512] per token, then rope,
            # write bf16 to the SBUF/DRAM destination AP. For q, 1/sqrt(HD)
            # and the head gain fold into the scale.
            def rms_rope_store(ps, ssp, cs, isl, dst, gain_ap, via_dma):
                sq = tmpp.tile([128, 512], BF, tag="rr_sq")
                nc.scalar.square(sq, ps)
                ss = ssp.tile([1, 512], F32, tag="rr_ss")
                nc.tensor.matmul(ss, lhsT=ones, rhs=sq, start=True, stop=True)
                scl = tmpp.tile([1, 512], F32, tag="rr_scl")
                if gain_ap is not None:
                    nc.scalar.activation(scl, ss, AF.Sqrt, bias=epsq[0:1, 0:1])
                    nc.vector.reciprocal(scl, scl)
                    nc.vector.tensor_scalar_mul(scl, in0=scl, scalar1=gain_ap)
                else:
                    nc.scalar.activation(scl, ss, AF.Sqrt, bias=epsk[0:1, 0:1],
                                         scale=1.0 / HD)
                    nc.vector.reciprocal(scl, scl)
                sclb = tmpp.tile([128, 512], F32, tag="rr_sclb")
                nc.gpsimd.partition_broadcast(sclb, scl)
                qn = tmpp.tile([128, 512], F32, tag="rr_qn")
                nc.vector.tensor_mul(qn, ps, sclb)
                qnsw = tmpp.tile([128, 512], F32, tag="rr_qnsw")
                nc.sync.dma_start(out=qnsw[0:64], in_=qn[64:128])
                nc.sync.dma_start(out=qnsw[64:128], in_=qn[0:64])
                t12 = tmpp.tile([128, 512], F32, tag="rr_t12")
                nc.vector.tensor_mul(t12, qn, cs[:, 0, isl])
                t34 = tmpp.tile([128, 512], F32, tag="rr_t34")
                nc.vector.tensor_mul(t34, qnsw, cs[:, 1, isl])
                if via_dma:
                    qr = tmpp.tile([128, 512], BF, tag="rr_qr")
                    nc.vector.tensor_add(qr, t12, t34)
                    nc.sync.dma_start(out=dst, in_=qr)
                else:
                    nc.vector.tensor_add(dst, t12, t34)

            with tc.tile_pool(name="wkv", bufs=1) as wkvp, \
                 tc.tile_pool(name="ck", bufs=1) as ckp:
                # prefetch K/V weights + key rope tables during phase Q
                wk = wkvp.tile([128, DT, HKV * HD], BF)
                nc.sync.dma_start(out=wk, in_=WkT.rearrange("(dt p) c -> p dt c", p=128))
                wv = wkvp.tile([128, DT, HKV * HD], BF)
                nc.sync.dma_start(out=wv, in_=WvT.rearrange("(dt p) c -> p dt c", p=128))
                ck = ckp.tile([128, 2, T], F32)
                nc.sync.dma_start(out=ck[:, 0], in_=cosk[:, :])
                nc.sync.dma_start(out=ck[:, 1], in_=sink[:, :])

                # ---------------- Phase Q ----------------
                with tc.tile_pool(name="xq", bufs=1) as xqp, \
                     tc.tile_pool(name="wq", bufs=2) as wqp, \
                     tc.tile_pool(name="psq", bufs=2, space="PSUM") as psp, \
                     tc.tile_pool(name="ssq", bufs=2, space="PSUM") as ssp, \
                     tc.tile_pool(name="cq", bufs=1) as cqp:
                    xq = xqp.tile([128, DT, R], BF)
                    xq_src = xqT.rearrange("(dt p) i -> p dt i", p=128)
                    for ch in range(4):   # chunked so the first matmuls start early
                        nc.sync.dma_start(out=xq[:, ch * 4:(ch + 1) * 4, :],
                                          in_=xq_src[:, ch * 4:(ch + 1) * 4, :])
                    cq = cqp.tile([128, 2, R], F32)
                    nc.sync.dma_start(out=cq[:, 0], in_=cosq[:, :])
                    nc.sync.dma_start(out=cq[:, 1], in_=sinq[:, :])
                    for hg in range(8):   # 2 heads per weight group
                        wq = wqp.tile([128, DT, 2 * HD], BF, tag="wq")
                        nc.sync.dma_start(
                            out=wq,
                            in_=WqT.rearrange("(dt p) c -> p dt c", p=128)[
                                :, :, hg * 256:(hg + 1) * 256])
                        for hh in range(2):
                            h = hg * 2 + hh
                            for ib in range(2):
                                isl = slice(ib * 512, (ib + 1) * 512)
                                ps = psp.tile([128, 512], F32, tag="q_ps")
                                for dt_ in range(DT):
                                    nc.tensor.matmul(
                                        ps,
                                        lhsT=wq[:, dt_, hh * HD:(hh + 1) * HD],
                                        rhs=xq[:, dt_, isl],
                                        start=(dt_ == 0), stop=(dt_ == DT - 1))
                                rms_rope_store(ps, ssp, cq, isl,
                                               qTd[h, :, isl], qg[0:1, h:h + 1],
                                               via_dma=True)

                # ---------------- Phase K/V ----------------
                with tc.tile_pool(name="xt", bufs=2) as xtp, \
                     tc.tile_pool(name="pskv", bufs=2, space="PSUM") as psp, \
                     tc.tile_pool(name="sskv", bufs=2, space="PSUM") as ssp:
                    for jb in range(4):
                        jsl = slice(jb * 512, (jb + 1) * 512)
                        xt = xtp.tile([128, DT, 512], BF, tag="xt")
                        nc.sync.dma_start(
                            out=xt,
                            in_=xT.rearrange("(dt p) t -> p dt t", p=128)[:, :, jsl])
                        for g in range(HKV):
                            ps = psp.tile([128, 512], F32, tag="k_ps")
                            for dt_ in range(DT):
                                nc.tensor.matmul(
                                    ps,
                                    lhsT=wk[:, dt_, g * HD:(g + 1) * HD],
                                    rhs=xt[:, dt_, :],
                                    start=(dt_ == 0), stop=(dt_ == DT - 1))
                            rms_rope_store(ps, ssp, ck, jsl,
                                           kg_all[:, g, jsl], None, via_dma=False)
                        for jt in range(4):
                            psv = psp.tile([128, 512], F32, tag="v_ps")
                            for dt_ in range(DT):
                                nc.tensor.matmul(
                                    psv,
                                    lhsT=xt[:, dt_, jt * 128:(jt + 1) * 128],
                                    rhs=wv[:, dt_, :],
                                    start=(dt_ == 0), stop=(dt_ == DT - 1))
                            nc.vector.tensor_copy(v_all[:, jb * 4 + jt, :], psv)

            # ---------------- Phase attention + proj ----------------
            with tc.tile_pool(name="yall", bufs=1) as yap:
                y_all = yap.tile([128, H, R], BF)   # [hd, h, i]
                with tc.tile_pool(name="mk", bufs=1) as mp, \
                     tc.tile_pool(name="qh", bufs=2) as qp, \
                     tc.tile_pool(name="pst", bufs=3, space="PSUM") as pstp, \
                     tc.tile_pool(name="psy", bufs=2, space="PSUM") as psyp, \
                     tc.tile_pool(name="pss", bufs=2, space="PSUM") as pssp, \
                     tc.tile_pool(name="pt", bufs=4) as ptp:
                    m0s = mp.tile([128, 8, RB], BF)
                    nc.sync.dma_start(out=m0s,
                                      in_=m0.rearrange("(jt p) i -> p jt i", p=128))
                    m1s = mp.tile([128, 8, RB], BF)
                    nc.sync.dma_start(out=m1s,
                                      in_=m1.rearrange("(jt p) i -> p jt i", p=128))
                    for g in range(HKV):
                        for hh in range(REP):
                            h = g * REP + hh
                            qh = qp.tile([128, R], BF, tag="qh")
                            nc.sync.dma_start(out=qh, in_=qTd[h])
                            for s in range(NSLOT):
                                njt = EXT[s] // 128
                                isl = slice(s * 512, (s + 1) * 512)
                                ys = psyp.tile([128, 512], F32, tag="ys")
                                ssum = pssp.tile([1, 512], F32, tag="ssA")
                                for jt in range(njt):
                                    st = pstp.tile([128, 512], F32, tag="st")
                                    nc.tensor.matmul(
                                        st,
                                        lhsT=kg_all[:, g, jt * 128:(jt + 1) * 128],
                                        rhs=qh[:, isl],
                                        start=True, stop=True)
                                    pe = ptp.tile([128, 512], BF, tag="pe")
                                    nc.scalar.activation(pe, st, AF.Exp)
                                    if s == 0:
                                        pt = ptp.tile([128, 512], BF, tag="pt")
                                        nc.vector.tensor_mul(pt, pe, m0s[:, jt, :])
                                    elif jt >= 8:
                                        pt = ptp.tile([128, 512], BF, tag="pt")
                                        nc.vector.tensor_mul(pt, pe, m1s[:, jt - 8, :])
                                    else:
                                        pt = pe
                                    nc.tensor.matmul(
                                        ssum, lhsT=ones, rhs=pt,
                                        start=(jt == 0), stop=(jt == njt - 1))
                                    nc.tensor.matmul(
                                        ys, lhsT=v_all[:, jt, g * HD:(g + 1) * HD],
                                        rhs=pt,
                                        start=(jt == 0), stop=(jt == njt - 1))
                                rc = tmpp.tile([1, 512], F32, tag="rc")
                                nc.vector.reciprocal(rc, ssum)
                                rcb = tmpp.tile([128, 512], F32, tag="rcb")
                                nc.gpsimd.partition_broadcast(rcb, rc)
                                nc.vector.tensor_mul(y_all[:, h, isl], ys, rcb)

                with tc.tile_pool(name="wp", bufs=2) as wpp, \
                     tc.tile_pool(name="pso", bufs=2, space="PSUM") as psp:
                    for og in range(4):
                        wp = wpp.tile([128, DT, 512], BF, tag="wp")
                        nc.sync.dma_start(
                            out=wp,
                            in_=WpT.rearrange("(ct p) o -> p ct o", p=128)[
                                :, :, og * 512:(og + 1) * 512])
                        for oo in range(4):
                            ot = og * 4 + oo
                            for ib in range(2):
                                isl = slice(ib * 512, (ib + 1) * 512)
                                ps = psp.tile([128, 512], F32, tag="o_ps")
                                for ct in range(DT):
                                    nc.tensor.matmul(
                                        ps,
                                        lhsT=wp[:, ct, oo * 128:(oo + 1) * 128],
                                        rhs=y_all[:, ct, isl],
                                        start=(ct == 0), stop=(ct == DT - 1))
                                ob = tmpp.tile([128, 512], F32, tag="ob")
                                nc.vector.tensor_copy(ob, ps)
                                nc.sync.dma_start(
                                    out=outT[ot * 128:(ot + 1) * 128, isl], in_=ob)

    nc.compile()
    return nc


def _rope_tables():
    inv = (1.0 / (np.float32(ROPE_BASE)
                  ** (np.arange(0, HD, 2, dtype=np.float32) / np.float32(HD))))
    t = np.arange(T, dtype=np.float32)
    freqs = np.outer(t, inv).astype(np.float32)          # [T, 64]
    c, si = np.cos(freqs).T, np.sin(freqs).T             # [64, T]
    # rows 0..63 twice for cos; +sin rows then -sin rows: with qn-halves
    # swapped this computes (q1*c + q2*s, q2*c - q1*s) in aligned DVE ops.
    cos_full = np.ascontiguousarray(np.concatenate([c, c], axis=0))
    sin_signed = np.ascontiguousarray(np.concatenate([si, -si], axis=0))
    return cos_full, sin_signed


def _masks(half):
    """{0,1} key-major masks for this core half (bf16). Returns (m0, m1)."""
    jj = np.arange(EXT[0], dtype=np.int64)[:, None]
    ii = np.arange(RB, dtype=np.int64)[None, :]
    # slot0 abs block: half0 -> 0 (rows 0..511), half1 -> 1 (rows 512..1023)
    off0 = 0 if half == 0 else RB
    m0 = (jj <= off0 + ii).astype(BF16)
    # slot1 vs keys 1024+jj: half0 -> abs 3 (rows 1536..), half1 -> abs 2 (1024..)
    off1 = RB if half == 0 else 0
    m1 = (jj <= off1 + ii).astype(BF16)
    return m0, m1


def _qrows(half):
    # (slot0 abs block, slot1 abs block) row offsets
    return (0, 3 * RB) if half == 0 else (RB, 2 * RB)


def kernel(**inputs):
    from concourse.bass_utils import run_bass_kernel_spmd

    x = np.ascontiguousarray(np.asarray(inputs["x"], dtype=np.float32))
    Wq = np.asarray(inputs["Wq"], dtype=np.float32)
    Wk = np.asarray(inputs["Wk"], dtype=np.float32)
    Wv = np.asarray(inputs["Wv"], dtype=np.float32)
    Wproj = np.asarray(inputs["Wproj"], dtype=np.float32)
    q_gain = np.ascontiguousarray(np.asarray(inputs["q_gain"], dtype=np.float32))

    if "nc" not in _CACHE:
        _CACHE["nc"] = _build()
    nc = _CACHE["nc"]

    def tb(a):  # transpose + bf16, contiguous
        return np.ascontiguousarray(a.T.astype(BF16))

    WqT = tb(Wq)
    WkT = tb(Wk)
    WvT = tb(Wv)
    WpT = tb(Wproj)
    cosT, sinT = _rope_tables()

    in_maps = []
    for c in range(8):
        b, half = divmod(c, 2)
        r0, r1 = _qrows(half)
        xb = x[b]
        xTc = tb(xb)
        xq = np.concatenate([xb[r0:r0 + RB], xb[r1:r1 + RB]], axis=0)
        xqT = tb(xq)
        cq = np.ascontiguousarray(
            np.concatenate([cosT[:, r0:r0 + RB], cosT[:, r1:r1 + RB]], axis=1))
        sq = np.ascontiguousarray(
            np.concatenate([sinT[:, r0:r0 + RB], sinT[:, r1:r1 + RB]], axis=1))
        m0, m1 = _masks(half)
        in_maps.append({
            "xT": xTc, "xqT": xqT, "WqT": WqT, "WkT": WkT, "WvT": WvT,
            "WpT": WpT, "qgain": q_gain, "cosq": cq, "sinq": sq,
            "cosk": cosT, "sink": sinT, "m0": m0, "m1": m1,
        })

    res = run_bass_kernel_spmd(nc, in_maps, core_ids=list(range(8)),
                               tmpdir=os.environ.get("BASS_KERNEL_TMPDIR"))
    _CACHE["res"] = res

    out = np.empty((B, T, DIM), dtype=np.float32)
    for c in range(8):
        b, half = divmod(c, 2)
        r0, r1 = _qrows(half)
        oT = res.results[c]["outT"]
        out[b, r0:r0 + RB] = oT[:, 0:RB].T
        out[b, r1:r1 + RB] = oT[:, RB:R].T
    return out



# revision 14
# speedup vs baseline: 1.0534x; 1.0534x over previous
"""Causal self-attention (GQA + RMSNorm + RoPE) Trainium2 Bass kernel.

Sharding: data-parallel over (batch, q-rows). 8 cores = 4 batches x 2 row
sets. Each core computes full K/V for its batch and 1024 q rows chosen as
8 x 128-row tiles: core half 0 takes even tiles, half 1 odd tiles. Tiles
are processed in descending causal-extent order so that a single
compile-time key-extent schedule E = (16,14,12,10,8,6,4,2) (in 128-key
tiles) is an upper bound for both halves: total scored coverage is 72
units/head vs 68 ideal causal, vs 96 dense-halves. No collectives.

On-chip layout is channel-major: scores are computed key-major
(S^T tile = K_tile^T.T @ Q^T) in [128,8,128] PSUM strips, exp'd in one
wide ACT instruction per strip, causal-masked by a {0,1} multiply on only
the last two key tiles (diagonal triangle + optional padding), and
consumed by per-q-tile accumulating ys (V^T @ P) and ssum (1^T @ P)
matmul chains. Normalization (1/ssum) is applied to the y tile.

The whole pipeline is software-pipelined one (head, q-tile) step deep so
the PE never waits on ACT exp / DVE mask latency. Q stays SBUF-resident
between phases (no DRAM round-trip). All matmul operands bf16 (fp32
accumulate); softmax/statistics math fp32.
"""

import os
import sys

sys.path.insert(0, "/opt/trn_rl_repo")

import ml_dtypes
import numpy as np

DIM = 2048
H = 16
HKV = 4
HD = 128
REP = H // HKV
B = 4
T = 2048
R = 1024          # q rows per core
DT = DIM // 128   # 16 contraction tiles
NJT = T // 128    # 16 key tiles
EPROC = (16, 14, 12, 10, 8, 6, 4, 2)  # key-tile extent per q-tile slot
ROPE_BASE = 10000.0
EPS = float(np.finfo(np.float32).eps)
BF16 = ml_dtypes.bfloat16

_CACHE = {}


def _strips(e):
    """Split an extent into PSUM-strip chunk widths (max 8 key tiles)."""
    out = [8] * (e // 8)
    if e % 8:
        out.append(e % 8)
    return out


def _build():
    """Build + compile the SPMD Bass program (once per process)."""
    from concourse import bacc
    import concourse.mybir as mybir
    import concourse.tile as tile

    F32 = mybir.dt.float32
    BF = mybir.dt.bfloat16
    AF = mybir.ActivationFunctionType

    nc = bacc.Bacc("TRN2", target_bir_lowering=False, debug=False)

    xT = nc.dram_tensor("xT", [DIM, T], BF, kind="ExternalInput")
    xqT = nc.dram_tensor("xqT", [DIM, R], BF, kind="ExternalInput")
    WqT = nc.dram_tensor("WqT", [DIM, DIM], BF, kind="ExternalInput")
    WkT = nc.dram_tensor("WkT", [DIM, HKV * HD], BF, kind="ExternalInput")
    WvT = nc.dram_tensor("WvT", [DIM, HKV * HD], BF, kind="ExternalInput")
    WpT = nc.dram_tensor("WpT", [DIM, DIM], BF, kind="ExternalInput")
    qgain = nc.dram_tensor("qgain", [H], F32, kind="ExternalInput")
    cosq = nc.dram_tensor("cosq", [HD, R], F32, kind="ExternalInput")
    sinq = nc.dram_tensor("sinq", [HD, R], F32, kind="ExternalInput")
    cosk = nc.dram_tensor("cosk", [HD, T], F32, kind="ExternalInput")
    sink = nc.dram_tensor("sink", [HD, T], F32, kind="ExternalInput")
    # per-core {0,1} mask for the last two key tiles of every q-tile strip:
    # half0 -> [tri, 0], half1 -> [1, tri]  (key-major [key, 2, row])
    mq = nc.dram_tensor("mq", [128, 2 * 128], BF, kind="ExternalInput")
    outT = nc.dram_tensor("outT", [DIM, R], F32, kind="ExternalOutput")

    with tile.TileContext(nc) as tc:
        with tc.tile_pool(name="const", bufs=1) as constp, \
             tc.tile_pool(name="res", bufs=1) as resp:
            ones = constp.tile([128, 1], BF)
            nc.vector.memset(ones, 1.0)
            qg = constp.tile([1, H], F32)
            nc.sync.dma_start(out=qg, in_=qgain[None, :])
            epsq = constp.tile([1, 1], F32)
            nc.vector.memset(epsq, EPS * HD)   # q scale: 1/sqrt(ssum + HD*eps)
            epsk = constp.tile([1, 1], F32)
            nc.vector.memset(epsk, EPS)        # k scale: rsqrt(ssum/HD + eps)

            # SBUF residents across phases
            q_all = resp.tile([128, H, R], BF)        # [hd, h, row]
            kg_all = resp.tile([128, HKV, T], BF)     # [hd, g, key]
            v_all = resp.tile([128, NJT, HKV * HD], BF)  # [key%128, kt, c]
            y_all = resp.tile([128, H, R], BF)        # [hd, h, row]

            # rms-normalize PSUM tile [128,512] per token, rope, write bf16
            # to dst AP. For q, 1/sqrt(HD) and head gain fold into the scale.
            def rms_rope_store(tmpp, ps, ssp, cs, isl, dst, gain_ap):
                sq = tmpp.tile([128, 512], BF, tag="rr_sq")
                nc.scalar.square(sq, ps)
                ss = ssp.tile([1, 512], F32, tag="rr_ss")
                nc.tensor.matmul(ss, lhsT=ones, rhs=sq, start=True, stop=True)
                scl = tmpp.tile([1, 512], F32, tag="rr_scl")
                if gain_ap is not None:
                    nc.scalar.activation(scl, ss, AF.Sqrt, bias=epsq[0:1, 0:1])
                    nc.vector.reciprocal(scl, scl)
                    nc.vector.tensor_scalar_mul(scl, in0=scl, scalar1=gain_ap)
                else:
                    nc.scalar.activation(scl, ss, AF.Sqrt, bias=epsk[0:1, 0:1],
                                         scale=1.0 / HD)
                    nc.vector.reciprocal(scl, scl)
                sclb = tmpp.tile([128, 512], F32, tag="rr_sclb")
                nc.gpsimd.partition_broadcast(sclb, scl)
                qn = tmpp.tile([128, 512], F32, tag="rr_qn")
                nc.vector.tensor_mul(qn, ps, sclb)
                qnsw = tmpp.tile([128, 512], F32, tag="rr_qnsw")
                nc.sync.dma_start(out=qnsw[0:64], in_=qn[64:128])
                nc.sync.dma_start(out=qnsw[64:128], in_=qn[0:64])
                t12 = tmpp.tile([128, 512], F32, tag="rr_t12")
                nc.vector.tensor_mul(t12, qn, cs[:, 0, isl])
                t34 = tmpp.tile([128, 512], F32, tag="rr_t34")
                nc.vector.tensor_mul(t34, qnsw, cs[:, 1, isl])
                nc.vector.tensor_add(dst, t12, t34)

            # ---------------- Phase Q ----------------
            rms_ctx = tc.tile_pool(name="rms", bufs=2)
            tmpp = rms_ctx.__enter__()
            with tc.tile_pool(name="xq", bufs=1) as xqp, \
                 tc.tile_pool(name="wq", bufs=2) as wqp, \
                 tc.tile_pool(name="psq", bufs=2, space="PSUM") as psp, \
                 tc.tile_pool(name="ssq", bufs=2, space="PSUM") as ssp, \
                 tc.tile_pool(name="cq", bufs=1) as cqp:
                xq = xqp.tile([128, DT, R], BF)
                xq_src = xqT.rearrange("(dt p) i -> p dt i", p=128)
                for ch in range(4):   # chunked so the first matmuls start early
                    nc.sync.dma_start(out=xq[:, ch * 4:(ch + 1) * 4, :],
                                      in_=xq_src[:, ch * 4:(ch + 1) * 4, :])
                cq = cqp.tile([128, 2, R], F32)
                nc.sync.dma_start(out=cq[:, 0], in_=cosq[:, :])
                nc.sync.dma_start(out=cq[:, 1], in_=sinq[:, :])
                prev = None
                for hg in range(8):   # 2 heads per weight group
                    wq = wqp.tile([128, DT, 2 * HD], BF, tag="wq")
                    nc.sync.dma_start(
                        out=wq,
                        in_=WqT.rearrange("(dt p) c -> p dt c", p=128)[
                            :, :, hg * 256:(hg + 1) * 256])
                    for hh in range(2):
                        h = hg * 2 + hh
                        for ib in range(2):
                            isl = slice(ib * 512, (ib + 1) * 512)
                            ps = psp.tile([128, 512], F32, tag="q_ps")
                            for dt_ in range(DT):
                                nc.tensor.matmul(
                                    ps,
                                    lhsT=wq[:, dt_, hh * HD:(hh + 1) * HD],
                                    rhs=xq[:, dt_, isl],
                                    start=(dt_ == 0), stop=(dt_ == DT - 1))
                            if prev is not None:
                                rms_rope_store(*prev)
                            prev = (tmpp, ps, ssp, cq, isl,
                                    q_all[:, h, isl], qg[0:1, h:h + 1])
                rms_rope_store(*prev)

            # ---------------- Phase K/V ----------------
            with tc.tile_pool(name="wkv", bufs=1) as wkvp, \
                 tc.tile_pool(name="ck", bufs=1) as ckp, \
                 tc.tile_pool(name="xt", bufs=2) as xtp, \
                 tc.tile_pool(name="pskv", bufs=2, space="PSUM") as psp, \
                 tc.tile_pool(name="sskv", bufs=2, space="PSUM") as ssp:
                wk = wkvp.tile([128, DT, HKV * HD], BF)
                nc.sync.dma_start(out=wk, in_=WkT.rearrange("(dt p) c -> p dt c", p=128))
                wv = wkvp.tile([128, DT, HKV * HD], BF)
                nc.sync.dma_start(out=wv, in_=WvT.rearrange("(dt p) c -> p dt c", p=128))
                ck = ckp.tile([128, 2, T], F32)
                nc.sync.dma_start(out=ck[:, 0], in_=cosk[:, :])
                nc.sync.dma_start(out=ck[:, 1], in_=sink[:, :])
                prev = None   # ('k', rms args) or ('v', copy args)
                for jb in range(4):
                    jsl = slice(jb * 512, (jb + 1) * 512)
                    xt = xtp.tile([128, DT, 512], BF, tag="xt")
                    nc.sync.dma_start(
                        out=xt,
                        in_=xT.rearrange("(dt p) t -> p dt t", p=128)[:, :, jsl])
                    for g in range(HKV):
                        ps = psp.tile([128, 512], F32, tag="k_ps")
                        for dt_ in range(DT):
                            nc.tensor.matmul(
                                ps,
                                lhsT=wk[:, dt_, g * HD:(g + 1) * HD],
                                rhs=xt[:, dt_, :],
                                start=(dt_ == 0), stop=(dt_ == DT - 1))
                        if prev is not None:
                            kind, args = prev
                            if kind == 'k':
                                rms_rope_store(*args)
                            else:
                                nc.vector.tensor_copy(args[0], args[1])
                        prev = ('k', (tmpp, ps, ssp, ck, jsl,
                                      kg_all[:, g, jsl], None))
                    for jt in range(4):
                        psv = psp.tile([128, 512], F32, tag="v_ps")
                        for dt_ in range(DT):
                            nc.tensor.matmul(
                                psv,
                                lhsT=xt[:, dt_, jt * 128:(jt + 1) * 128],
                                rhs=wv[:, dt_, :],
                                start=(dt_ == 0), stop=(dt_ == DT - 1))
                        if prev is not None:
                            kind, args = prev
                            if kind == 'k':
                                rms_rope_store(*args)
                            else:
                                nc.vector.tensor_copy(args[0], args[1])
                        prev = ('v', (v_all[:, jb * 4 + jt, :], psv))
                kind, args = prev
                nc.vector.tensor_copy(args[0], args[1])
            rms_ctx.__exit__(None, None, None)

            # ---------------- Phase attention ----------------
            # One step = (head h, q-tile slot i). Steps are software-
            # pipelined: scores+exp+mask of step u are emitted before the
            # ys/ssum/normalize of step u-1, so the PE streams while ACT
            # exps the previous step's strips.
            with tc.tile_pool(name="mk", bufs=1) as mp, \
                 tc.tile_pool(name="pts", bufs=4) as ptp, \
                 tc.tile_pool(name="ptm", bufs=2) as ptmp, \
                 tc.tile_pool(name="ntp", bufs=2) as ntp, \
                 tc.tile_pool(name="sc", bufs=3, space="PSUM") as scp, \
                 tc.tile_pool(name="ys", bufs=2, space="PSUM") as ysp:
                mqs = mp.tile([128, 2, 128], BF)
                nc.sync.dma_start(out=mqs, in_=mq.rearrange("p (t r) -> p t r", t=2))

                def emit_scores(g, h, i):
                    """Scores + exp + mask for step (h, i). Returns state
                    for the consume half."""
                    e = EPROC[i]
                    tsl = slice(i * 128, (i + 1) * 128)
                    pts = []           # (pt_tile, width, kt_base)
                    ptm = None
                    kt_base = 0
                    for w in _strips(e):
                        sp = scp.tile([128, 8, 128], F32, tag="sc")
                        for k in range(w):
                            kt = kt_base + k
                            nc.tensor.matmul(
                                sp[:, k, :],
                                lhsT=kg_all[:, g, kt * 128:(kt + 1) * 128],
                                rhs=q_all[:, h, tsl],
                                start=True, stop=True)
                        pt = ptp.tile([128, 8, 128], BF, tag="pt")
                        nc.scalar.activation(pt[:, 0:w, :], sp[:, 0:w, :], AF.Exp)
                        if kt_base + w == e:   # strip holds the last 2 kts
                            tl = (e - 2) - kt_base
                            ptm = ptmp.tile([128, 2, 128], BF, tag="ptm")
                            nc.vector.tensor_mul(ptm, pt[:, tl:tl + 2, :], mqs)
                        pts.append((pt, w, kt_base))
                        kt_base += w
                    return (g, h, i, e, pts, ptm)

                def emit_consume(st):
                    g, h, i, e, pts, ptm = st
                    tsl = slice(i * 128, (i + 1) * 128)
                    ys = ysp.tile([128, 512], F32, tag="ys")
                    for pt, w, kt_base in pts:
                        for k in range(w):
                            kt = kt_base + k
                            src = (ptm[:, kt - (e - 2), :] if kt >= e - 2
                                   else pt[:, k, :])
                            nc.tensor.matmul(
                                ys[:, 0:128],
                                lhsT=v_all[:, kt, g * HD:(g + 1) * HD],
                                rhs=src,
                                start=(kt == 0), stop=(kt == e - 1))
                    for pt, w, kt_base in pts:
                        for k in range(w):
                            kt = kt_base + k
                            src = (ptm[:, kt - (e - 2), :] if kt >= e - 2
                                   else pt[:, k, :])
                            nc.tensor.matmul(
                                ys[0:1, 256:384],
                                lhsT=ones, rhs=src,
                                start=(kt == 0), stop=(kt == e - 1))
                    rc = ntp.tile([1, 128], F32, tag="rc")
                    nc.vector.reciprocal(rc, ys[0:1, 256:384])
                    rcb = ntp.tile([128, 128], F32, tag="rcb")
                    nc.gpsimd.partition_broadcast(rcb, rc)
                    nc.vector.tensor_mul(y_all[:, h, tsl], ys[:, 0:128], rcb)

                prev = None
                for g in range(HKV):
                    for hh in range(REP):
                        h = g * REP + hh
                        for i in range(8):
                            st = emit_scores(g, h, i)
                            if prev is not None:
                                emit_consume(prev)
                            prev = st
                emit_consume(prev)

            # ---------------- Phase proj ----------------
            with tc.tile_pool(name="wp", bufs=2) as wpp, \
                 tc.tile_pool(name="obp", bufs=2) as obp, \
                 tc.tile_pool(name="pso", bufs=2, space="PSUM") as psp:
                for og in range(4):
                    wp = wpp.tile([128, DT, 512], BF, tag="wp")
                    nc.sync.dma_start(
                        out=wp,
                        in_=WpT.rearrange("(ct p) o -> p ct o", p=128)[
                            :, :, og * 512:(og + 1) * 512])
                    for oo in range(4):
                        ot = og * 4 + oo
                        for ib in range(2):
                            isl = slice(ib * 512, (ib + 1) * 512)
                            ps = psp.tile([128, 512], F32, tag="o_ps")
                            for ct in range(DT):
                                nc.tensor.matmul(
                                    ps,
                                    lhsT=wp[:, ct, oo * 128:(oo + 1) * 128],
                                    rhs=y_all[:, ct, isl],
                                    start=(ct == 0), stop=(ct == DT - 1))
                            ob = obp.tile([128, 512], F32, tag="ob")
                            nc.vector.tensor_copy(ob, ps)
                            nc.sync.dma_start(
                                out=outT[ot * 128:(ot + 1) * 128, isl], in_=ob)

    nc.compile()
    return nc


def _rope_tables():
    inv = (1.0 / (np.float32(ROPE_BASE)
                  ** (np.arange(0, HD, 2, dtype=np.float32) / np.float32(HD))))
    t = np.arange(T, dtype=np.float32)
    freqs = np.outer(t, inv).astype(np.float32)          # [T, 64]
    c, si = np.cos(freqs).T, np.sin(freqs).T             # [64, T]
    # rows 0..63 twice for cos; +sin rows then -sin rows: with qn-halves
    # swapped this computes (q1*c + q2*s, q2*c - q1*s) in aligned DVE ops.
    cos_full = np.ascontiguousarray(np.concatenate([c, c], axis=0))
    sin_signed = np.ascontiguousarray(np.concatenate([si, -si], axis=0))
    return cos_full, sin_signed


def _proc_tiles(half):
    """q-tile (128-row block) indices in processing order: extent of slot
    i must be <= EPROC[i]."""
    return [e - 2 for e in EPROC] if half == 0 else [e - 1 for e in EPROC]


def _mask(half):
    """[128 key, 2, 128 row] {0,1} bf16 mask for the last 2 key tiles of
    every strip: half0 -> [tri, 0], half1 -> [1, tri]."""
    jj = np.arange(128)[:, None]
    rr = np.arange(128)[None, :]
    tri = (jj <= rr).astype(BF16)
    m = np.zeros((128, 2, 128), dtype=BF16)
    if half == 0:
        m[:, 0] = tri
    else:
        m[:, 0] = 1
        m[:, 1] = tri
    return np.ascontiguousarray(m.reshape(128, 256))


def kernel(**inputs):
    from concourse.bass_utils import run_bass_kernel_spmd

    x = np.ascontiguousarray(np.asarray(inputs["x"], dtype=np.float32))
    Wq = np.asarray(inputs["Wq"], dtype=np.float32)
    Wk = np.asarray(inputs["Wk"], dtype=np.float32)
    Wv = np.asarray(inputs["Wv"], dtype=np.float32)
    Wproj = np.asarray(inputs["Wproj"], dtype=np.float32)
    q_gain = np.ascontiguousarray(np.asarray(inputs["q_gain"], dtype=np.float32))

    if "nc" not in _CACHE:
        _CACHE["nc"] = _build()
    nc = _CACHE["nc"]

    def tb(a):  # transpose + bf16, contiguous
        return np.ascontiguousarray(a.T.astype(BF16))

    WqT = tb(Wq)
    WkT = tb(Wk)
    WvT = tb(Wv)
    WpT = tb(Wproj)
    cosT, sinT = _rope_tables()

    in_maps = []
    for c in range(8):
        b, half = divmod(c, 2)
        tiles = _proc_tiles(half)
        ridx = np.concatenate([np.arange(t * 128, (t + 1) * 128) for t in tiles])
        xb = x[b]
        in_maps.append({
            "xT": tb(xb),
            "xqT": tb(xb[ridx]),
            "WqT": WqT, "WkT": WkT, "WvT": WvT, "WpT": WpT,
            "qgain": q_gain,
            "cosq": np.ascontiguousarray(cosT[:, ridx]),
            "sinq": np.ascontiguousarray(sinT[:, ridx]),
            "cosk": cosT, "sink": sinT,
            "mq": _mask(half),
        })

    res = run_bass_kernel_spmd(nc, in_maps, core_ids=list(range(8)),
                               tmpdir=os.environ.get("BASS_KERNEL_TMPDIR"))
    _CACHE["res"] = res

    out = np.empty((B, T, DIM), dtype=np.float32)
    for c in range(8):
        b, half = divmod(c, 2)
        oT = res.results[c]["outT"]
        for i, t in enumerate(_proc_tiles(half)):
            out[b, t * 128:(t + 1) * 128] = oT[:, i * 128:(i + 1) * 128].T
    return out
